# revision 40
# baseline (speedup 1.0000x reference)
"""Multihead attention (custom freq-bias) Trainium2 Bass kernel — v3.

Full inputs -> shard across 8 NeuronCores -> SPMD bass kernel -> host combine.

Sharding: core c handles batch b = c//2 and head-half s = c%2 (8 of 16 heads).
Heads are processed in 4 PAIRS per core.

v3 is a software-pipelined emission built around ScalarE (exp) as the pacing
engine (~273us of ACTIVATE at N=1024):

  - The MAIN stream is (pair, chunk of 4 kt, qh-pass, kt) "units": row-tiled
    QK^T matmul pairs into psa/psb [128,1024] PSUM + 2 exp ACTIVATEs writing
    et[kt] [128, 4096] bf16 (head A cols 0:2048, head B 2048:4096), plus the
    DVE collapse chain es += et[kt] emitted on qh=1 passes.
  - ALL other PE work is emitted as fine-grained FILLER quanta between units
    so TensorE stays dense (HAM warm) and ScalarE never starves:
      * qk projections: LDW amortized over 2 N=512 MMs per stationary w chunk
      * v projection: row-tiled K=64 halves into the two banks of one
        [128,1024] prj PSUM tile (hides LDWEIGHTS), summed during the drain
      * AV: col-tiled head pairs accumulating 4 kt per psy [128,512] visit
      * denominators: col-tiled ones-matmuls (head A -> psd[0:64], head B ->
        psd[64:128]) + one reciprocal + one normalize mul per (pair, qc)
      * out_proj: per-PAIR partials (4 DRAM partials, host sums) so the tail
        after the last exp is only pair 3's den/norm/out_proj.
  - Urgent fillers (AV, den/norm) jump the queue; background fillers
    (projections, out_proj) run FIFO.
"""

import numpy as np
import ml_dtypes
from collections import deque

import concourse.bass as bass
import concourse.tile as tile
from concourse import bacc, mybir

F32 = mybir.dt.float32
BF16 = mybir.dt.bfloat16
AF = mybir.ActivationFunctionType
ADD = mybir.AluOpType.add
MULT = mybir.AluOpType.mult

B, N, C, H, D = 4, 2048, 1024, 16, 64
NCORES = 8
HC = C // 2          # 512 channels per core (8 heads x 64)
NKT = N // 128       # 16 key tiles
NPAIR = 4
NCH = 4              # chunks per pair
CKT = NKT // NCH     # kt per chunk (4)

FILLER_NS = 2000.0   # filler PE-ns budget pumped per (kt, qh) unit


def ts(i, sz):
    return slice(i * sz, (i + 1) * sz)


def kernel_body(ctx, tc, out, ins):
    """Per-core kernel. out: [4*2048, 1024] bf16 DRAM (4 per-pair partials)."""
    nc = tc.nc
    xq, xk, xv = ins["xqt"], ins["xkt"], ins["xvt"]      # [1024, 2048] bf16
    wq, wk, wv = ins["wq"], ins["wk"], ins["wv"]          # [1024, 512] bf16
    wp = ins["wp"]                                        # [512, 1024] bf16
    bqc, bkc = ins["bqc"], ins["bkc"]                     # [128, 4] f32
    bvb = ins["bvb"]                                      # [128, 512] bf16
    freq = ins["freq"]                                    # [128, 16] f32
    singles = ctx.enter_context(tc.tile_pool(name="singles", bufs=1))

    # ---- persistent SBUF residents ----
    yT = [singles.tile([128, N], BF16, name=f"yT{p}") for p in range(NPAIR)]
    vt = [singles.tile([128, HC], BF16, name=f"v{i}") for i in range(NKT)]
    wp_sb = [singles.tile([128, C], BF16, name=f"wp{p}") for p in range(NPAIR)]
    wv_sb = [singles.tile([128, HC], BF16, name=f"wv{k}") for k in range(8)]
    bqc_sb = singles.tile([128, 4], F32, name="bqc")
    bkc_sb = singles.tile([128, 4], F32, name="bkc")
    bvb_sb = singles.tile([128, HC], BF16, name="bvb")
    freq_sb = singles.tile([128, NKT], F32, name="freq")
    ones64 = singles.tile([128, 64], BF16, name="ones64")
    warm = singles.tile([1, 8], F32, name="warm")

    # only the tiny tensors up-front; wv/bvb/wp DMAs are deferred into the
    # v_proj / out_proj prefetches so the prologue x loads go first
    nc.sync.dma_start(out=bqc_sb, in_=bqc)
    nc.sync.dma_start(out=bkc_sb, in_=bkc)
    nc.sync.dma_start(out=freq_sb, in_=freq)
    nc.vector.memset(ones64, 1.0)
    # warm up the exp table (ACT_TABLE_LOAD) off the critical path
    nc.vector.memset(warm, 0.0)
    nc.scalar.activation(out=warm, in_=warm, func=AF.Exp)

    # ---- pools ----
    qkpool = ctx.enter_context(tc.tile_pool(name="qkpool", bufs=2))
    xpool = ctx.enter_context(tc.tile_pool(name="xpool", bufs=10))
    wpool = ctx.enter_context(tc.tile_pool(name="wpool", bufs=32))
    etpool = ctx.enter_context(tc.tile_pool(name="etpool", bufs=9))
    espool = ctx.enter_context(tc.tile_pool(name="espool", bufs=2))
    otpool = ctx.enter_context(tc.tile_pool(name="otpool", bufs=3))
    rpool = ctx.enter_context(tc.tile_pool(name="rpool", bufs=2))
    psS = ctx.enter_context(tc.tile_pool(name="psS", bufs=1, space="PSUM"))
    psY = ctx.enter_context(tc.tile_pool(name="psY", bufs=2, space="PSUM"))
    psP = ctx.enter_context(tc.tile_pool(name="psP", bufs=1, space="PSUM"))

    # rotating per-pair tiles
    qT = {}   # pair -> tile [128, 2048]
    kT = {}
    et = {}   # kt -> tile [128, 4096], q-block-major [A512|B512] per block
    es = {}   # pair -> tile [128, 4096] (DVE partial: et[0..13])
    esg = {}  # pair -> tile [128, 4096] (GpSimd partial: et[14]+et[15])

    # =================================================================
    # filler generator machinery
    # =================================================================

    class Gen:
        """Filler work generator: .prefetch() emits DMAs; .step() emits one
        quantum and returns its est PE ns, or None when exhausted."""
        def __init__(self, name, prefetch_fn, quanta):
            self.name = name
            self._prefetch = prefetch_fn
            self._quanta = quanta  # list of (est_ns, fn)
            self._i = 0
            self.prefetched = False

        def prefetch(self):
            if not self.prefetched:
                self.prefetched = True
                if self._prefetch:
                    self._prefetch()

        def step(self):
            if self._i >= len(self._quanta):
                return None
            est, fn = self._quanta[self._i]
            self._i += 1
            self.prefetch()
            fn()
            return est

        def run_all(self):
            while self.step() is not None:
                pass

        def run_upto(self, n):
            while self._i < n and self.step() is not None:
                pass

    urgent = deque()
    background = deque()

    def pump(budget):
        while budget > 0:
            q = urgent if urgent else background
            if not q:
                return
            g = q[0]
            est = g.step()
            if est is None:
                q.popleft()
                # prefetch the next background gen's DMAs early
                if q is background and background:
                    background[0].prefetch()
                continue
            budget -= est

    # =================================================================
    # work emitters
    # =================================================================

    def gen_qk_proj(pair, which, qh):
        """q or k projection for one pair, one q-half -> qT/kT[pair] cols."""
        x_d, w_d, b_sb = ((xq, wq, bqc_sb) if which == "q" else (xk, wk, bkc_sb))
        dstmap = qT if which == "q" else kT
        x_sb = []
        w_sb = []

        # q-projection inputs load via the GpSimd DMA queue, k via Sync —
        # two queues in parallel halve the ramp-critical load time
        dma_eng = nc.gpsimd if which == "k" else nc.sync

        def prefetch():
            for k in range(8):
                t = xpool.tile([128, 1024], BF16, tag="x", name=f"x{which}{k}")
                dma_eng.dma_start(out=t, in_=x_d[ts(k, 128), ts(qh, 1024)])
                x_sb.append(t)
                tw = wpool.tile([128, 128], BF16, tag="w", name=f"w{which}{k}")
                dma_eng.dma_start(out=tw, in_=w_d[ts(k, 128), ts(pair, 128)])
                w_sb.append(tw)

        state = {}

        def mk_mm(k):
            def fn():
                if k == 0:
                    if pair not in dstmap:
                        dstmap[pair] = qkpool.tile(
                            [128, N], BF16, tag=which, name=f"{which}T{pair}")
                    state["ps"] = psP.tile([128, 1024], F32, tag="prj", name="psq")
                ps = state["ps"]
                for j in range(2):
                    nc.tensor.matmul(ps[:, ts(j, 512)], w_sb[k],
                                     x_sb[k][:, ts(j, 512)],
                                     start=(k == 0), stop=(k == 7))
            return fn

        def drain():
            nc.vector.tensor_scalar(
                dstmap[pair][:, ts(qh, 1024)], state["ps"],
                b_sb[:, pair:pair + 1], None, ADD)

        quanta = [(520.0, mk_mm(k)) for k in range(8)]
        quanta.append((0.0, drain))
        return Gen(f"qk{which}{pair}h{qh}", prefetch, quanta)

    def gen_v_proj(qh):
        """v projection for kt in [qh*8, qh*8+8): row-tiled K=64 halves into
        the two banks of one [128,1024] prj tile, summed+biased on drain."""
        x_sb = []

        def prefetch():
            if qh == 0:
                for k in range(8):
                    nc.sync.dma_start(out=wv_sb[k], in_=wv[ts(k, 128), :])
                nc.sync.dma_start(out=bvb_sb, in_=bvb)
            for k in range(8):
                t = xpool.tile([128, 1024], BF16, tag="x", name=f"xv{k}")
                nc.sync.dma_start(out=t, in_=xv[ts(k, 128), ts(qh, 1024)])
                x_sb.append(t)

        state = {}

        def mk_mm(ktl, khalf):
            def fn():
                if khalf == 0:
                    state["ps"] = psP.tile([128, 1024], F32, tag="prj", name="psv")
                ps = state["ps"]
                for k in (2 * khalf, 2 * khalf + 1):
                    xc = x_sb[k][:, ts(ktl, 128)]
                    nc.tensor.matmul(ps[:, 0:512], xc[0:64, :],
                                     wv_sb[k][0:64, :],
                                     start=(k == 0), stop=(k == 7),
                                     skip_group_check=True)
                    nc.tensor.matmul(ps[:, 512:1024], xc[64:128, :],
                                     wv_sb[k][64:128, :],
                                     start=(k == 0), stop=(k == 7),
                                     skip_group_check=True)
            return fn

        def mk_drain(ktl):
            def fn():
                kt = qh * 8 + ktl
                ps = state["ps"]
                # tensor_tensor may read only ONE input from PSUM -> 3 steps;
                # the SBUF-only bias add runs on the otherwise-idle GpSimd
                nc.vector.tensor_copy(vt[kt], ps[:, 0:512])
                nc.vector.tensor_tensor(vt[kt], vt[kt], ps[:, 512:1024], ADD)
                nc.gpsimd.tensor_add(vt[kt], vt[kt], bvb_sb)
            return fn

        quanta = []
        for ktl in range(8):
            for khalf in range(4):
                quanta.append((880.0, mk_mm(ktl, khalf)))
            quanta.append((0.0, mk_drain(ktl)))
        return Gen(f"v{qh}", prefetch, quanta)

    def gen_av(pair, chunk):
        """AV for one chunk of 8 kt: head A accumulates into psyA (its own
        bank, partitions 0:64), head B into psyB (partitions 64:128) — two
        banks so the col-tiled pair runs concurrently. Two half-drains into
        yT (copy for chunk 0, add for chunk 1)."""
        def mk_qc(qc):
            def fn():
                if qc == 0:
                    force_v(chunk)
                psy = psY.tile([128, 512], F32, tag="psy", name="psy")
                for i in range(CKT):
                    kt = chunk * CKT + i
                    nc.tensor.matmul(psy[0:64, :],
                                     vt[kt][:, pair * 128:pair * 128 + 64],
                                     et[kt][:, qc * 1024:qc * 1024 + 512],
                                     start=(i == 0), stop=(i == CKT - 1),
                                     skip_group_check=True)
                    nc.tensor.matmul(psy[64:128, :],
                                     vt[kt][:, pair * 128 + 64:pair * 128 + 128],
                                     et[kt][:, qc * 1024 + 512:(qc + 1) * 1024],
                                     start=(i == 0), stop=(i == CKT - 1),
                                     skip_group_check=True)
                if chunk == 0:
                    nc.vector.tensor_copy(yT[pair][:, ts(qc, 512)], psy)
                else:
                    nc.vector.tensor_tensor(yT[pair][:, ts(qc, 512)],
                                            yT[pair][:, ts(qc, 512)], psy, ADD)
            return fn
        return Gen(f"av{pair}c{chunk}", None,
                   [(1900.0, mk_qc(qc)) for qc in range(4)])

    def gen_den_norm(pair):
        """Denominators via col-tiled ones-matmuls over es + esg, then one
        reciprocal + one normalize mul per qc (norm on GpSimd for pairs 0-2)."""
        def mk_qc(qc):
            def fn():
                psd = psY.tile([128, 512], F32, tag="psy", name="psd")
                for half, lo in ((slice(0, 64), 0), (slice(64, 128), 512)):
                    cs = slice(qc * 1024 + lo, qc * 1024 + lo + 512)
                    two = False
                    nc.tensor.matmul(psd[half, :], ones64, es[pair][:, cs],
                                     start=True, stop=not two,
                                     skip_group_check=True)
                    if two:
                        nc.tensor.matmul(psd[half, :], ones64, esg[pair][:, cs],
                                         start=False, stop=True,
                                         skip_group_check=True)
                rsb = rpool.tile([128, 512], F32, tag="rsb", name="rsb")
                nc.vector.reciprocal_approx_fast(out=rsb, in_=psd)
                if pair < 3:
                    nc.gpsimd.tensor_mul(yT[pair][:, ts(qc, 512)],
                                         yT[pair][:, ts(qc, 512)], rsb)
                else:
                    nc.vector.tensor_tensor(yT[pair][:, ts(qc, 512)],
                                            yT[pair][:, ts(qc, 512)], rsb, MULT)
            return fn
        return Gen(f"den{pair}", None, [(900.0, mk_qc(qc)) for qc in range(4)])

    def gen_out_proj(group, pool, tags, use_scalar=False):
        """Group output partial (pairs 2g, 2g+1 accumulated in PSUM) ->
        DRAM rows [g*2048, (g+1)*2048)."""
        p0, p1 = 2 * group, 2 * group + 1

        def prefetch():
            for p in (p0, p1):
                nc.sync.dma_start(out=wp_sb[p], in_=wp[ts(p, 128), :])

        def mk_m(m):
            def fn():
                tag = tags[m % len(tags)]
                ps = pool.tile([128, 1024], F32, tag=tag, name="psO")
                for p in (p0, p1):
                    for n2 in range(2):
                        nc.tensor.matmul(ps[:, ts(n2, 512)], yT[p][:, ts(m, 128)],
                                         wp_sb[p][:, ts(n2, 512)],
                                         start=(p == p0), stop=(p == p1),
                                         skip_group_check=True)
                ot = otpool.tile([128, 1024], BF16, tag="ot", name="ot")
                if use_scalar and m % 2 == 0:
                    nc.scalar.copy(ot, ps)
                else:
                    nc.vector.tensor_copy(ot, ps)
                nc.gpsimd.dma_start(
                    out=out[group * N + m * 128:group * N + (m + 1) * 128, :],
                    in_=ot)
            return fn
        return Gen(f"op{group}", prefetch, [(1100.0, mk_m(m)) for m in range(16)])

    # =================================================================
    # main pipelined emission
    # =================================================================

    def unit(pair, kt, qh):
        """One QK^T + exp unit; also advances the es collapse chain on qh=1.

        et[kt] layout is q-block-major: block qb = qh*2+j (TRUE q-block of
        512) occupies cols [qb*1024, qb*1024+1024) as [headA 512 | headB 512].
        Each j gets ONE [128,1024] psum tile: head A -> bank 0, head B ->
        bank 1 (different banks so the row-tiled pair runs concurrently), and
        ONE N=1024 exp drains it."""
        for j in range(2):
            qb = qh * 2 + j
            ps = psS.tile([128, 1024], F32, tag=("sa" if j == 0 else "sb"),
                          name="ps")
            nc.tensor.matmul(ps[:, 0:512],
                             kT[pair][0:64, ts(kt, 128)],
                             qT[pair][0:64, ts(qb, 512)],
                             start=True, stop=True, skip_group_check=True)
            nc.tensor.matmul(ps[:, 512:1024],
                             kT[pair][64:128, ts(kt, 128)],
                             qT[pair][64:128, ts(qb, 512)],
                             start=True, stop=True, skip_group_check=True)
            nc.scalar.activation(out=et[kt][:, ts(qb, 1024)], in_=ps,
                                 func=AF.Exp, bias=freq_sb[:, kt:kt + 1],
                                 scale=1.0)
        if qh == 1:
            # es collapse: DVE chain over et[0..13]; the last two tiles go to
            # the otherwise-idle GpSimd as a separate partial (es_g), folded
            # back in by the denominator matmuls.
            dve_last = 15
            if kt == 1:
                es[pair] = espool.tile([128, 2 * N], BF16, tag="es", name="es")
                nc.vector.tensor_tensor(es[pair], et[0], et[1], ADD)
            elif 1 < kt <= dve_last:
                nc.vector.tensor_tensor(es[pair], es[pair], et[kt], ADD)


    # prologue: pair 0 qh0 projections run up front (ScalarE is idle anyway)
    g = gen_qk_proj(0, "q", 0)
    g.prefetch()
    gk = gen_qk_proj(0, "k", 0)
    gk.prefetch()
    g.run_all()
    gk.run_all()

    pair_qk_gens = {p: {0: [], 1: []} for p in range(NPAIR)}

    def enqueue_qk(pair):
        for qh in range(2):
            for which in ("q", "k"):
                g2 = gen_qk_proj(pair, which, qh)
                pair_qk_gens[pair][qh].append(g2)
                background.append(g2)

    def force_qk(pair, qh):
        for g2 in pair_qk_gens[pair][qh]:
            if g2 in background:
                background.remove(g2)
            g2.run_all()

    g01 = gen_qk_proj(0, "q", 1); gk1 = gen_qk_proj(0, "k", 1)
    pair_qk_gens[0][1] = [g01, gk1]
    v_gens = [gen_v_proj(0), gen_v_proj(1)]
    background.extend([g01, gk1, v_gens[0], v_gens[1]])
    enqueue_qk(1)
    background[0].prefetch()

    def force_v(chunk):
        """Ensure v_proj writes for this chunk's kt range are emitted before
        the AV generator that reads them (Tile deps follow emission order)."""
        kt_hi = chunk * CKT + CKT - 1
        for qh2 in range(2):
            lo, hi = qh2 * 8, qh2 * 8 + 7
            if kt_hi < lo:
                break
            n = (min(kt_hi, hi) - lo + 1) * 5
            v_gens[qh2].run_upto(n)

    for pair in range(NPAIR):
        # backstop: this pair's qh0 projections MUST be fully emitted before
        # its attention units (the pump normally finishes them as filler)
        force_qk(pair, 0)

        # enqueue next pair's projections / previous pairs' out_proj
        if pair == 1:
            enqueue_qk(2)
        elif pair == 2:
            background.append(gen_out_proj(0, psP, ["prj"]))
            enqueue_qk(3)

        for chunk in range(NCH):
            for qh in range(2):
                if qh == 1 and chunk == 0:
                    force_qk(pair, 1)
                for i in range(CKT):
                    kt = chunk * CKT + i
                    if qh == 0:
                        et[kt] = etpool.tile([128, 2 * N], BF16, tag="et",
                                             name=f"et{kt}")
                    unit(pair, kt, qh)
                    pump(FILLER_NS)
            urgent.append(gen_av(pair, chunk))
        urgent.append(gen_den_norm(pair))

    # tail: drain remaining urgent work (AV c1 + den/norm of pair 3), then
    # group 1's out_proj on the freed score-PSUM banks
    while urgent:
        if urgent[0].step() is None:
            urgent.popleft()
    gen_out_proj(1, psS, ["sa", "sb"], use_scalar=True).run_all()
    while background:
        if background[0].step() is None:
            background.popleft()


INPUT_SPECS = {
    "xqt": ([C, N], BF16), "xkt": ([C, N], BF16), "xvt": ([C, N], BF16),
    "wq": ([C, HC], BF16), "wk": ([C, HC], BF16), "wv": ([C, HC], BF16),
    "wp": ([HC, C], BF16),
    "bqc": ([128, 4], F32), "bkc": ([128, 4], F32),
    "bvb": ([128, HC], BF16),
    "freq": ([128, NKT], F32),
}


def build_nc():
    from contextlib import ExitStack
    nc = bacc.Bacc("TRN2", target_bir_lowering=False, debug=False)
    ins = {name: nc.dram_tensor(name, shape, dt, kind="ExternalInput").ap()
           for name, (shape, dt) in INPUT_SPECS.items()}
    out = nc.dram_tensor("out", [2 * N, C], BF16, kind="ExternalOutput").ap()
    with tile.TileContext(nc) as tc:
        with ExitStack() as ctx:
            kernel_body(ctx, tc, out, ins)
    nc.compile()
    return nc


def make_freq():
    fr = np.linspace(0.0, 1.0, N, dtype=np.float32)
    fb = -((fr - 0.5) ** 2) * 10.0
    return np.ascontiguousarray(fb.reshape(NKT, 128).T).astype(np.float32)


def make_shards(inputs):
    """Full inputs -> list of 8 per-core input dicts."""
    q = np.asarray(inputs["query"], np.float32)
    k = np.asarray(inputs["key"], np.float32)
    v = np.asarray(inputs["value"], np.float32)
    Wq = np.asarray(inputs["Wq"], np.float32); bq = np.asarray(inputs["bq"], np.float32)
    Wk = np.asarray(inputs["Wk"], np.float32); bk = np.asarray(inputs["bk"], np.float32)
    Wv = np.asarray(inputs["Wv"], np.float32); bv = np.asarray(inputs["bv"], np.float32)
    Wp = np.asarray(inputs["Wp"], np.float32)
    freq = make_freq()
    scale = np.float32(1.0 / np.sqrt(D))

    shards = []
    for c in range(NCORES):
        b, s = c // 2, c % 2
        cs = slice(s * HC, (s + 1) * HC)
        bq_s = (bq[cs] * scale).astype(np.float32)
        bk_s = bk[cs].astype(np.float32)
        sh = {
            "xqt": np.ascontiguousarray(q[b].T),
            "xkt": np.ascontiguousarray(k[b].T),
            "xvt": np.ascontiguousarray(v[b].T),
            "wq": np.ascontiguousarray(Wq[:, cs]) * scale,
            "wk": np.ascontiguousarray(Wk[:, cs]),
            "wv": np.ascontiguousarray(Wv[:, cs]),
            "wp": np.ascontiguousarray(Wp[cs, :]),
            "bqc": np.ascontiguousarray(bq_s.reshape(4, 128).T),
            "bkc": np.ascontiguousarray(bk_s.reshape(4, 128).T),
            "bvb": np.broadcast_to(bv[cs], (128, HC)).copy(),
            "freq": freq,
        }
        for kk, (shape, dt) in INPUT_SPECS.items():
            want = ml_dtypes.bfloat16 if dt == BF16 else np.float32
            sh[kk] = np.asarray(sh[kk]).astype(want)
        shards.append(sh)
    return shards


_NC_CACHE = None


def kernel(**inputs):
    global _NC_CACHE
    shards = make_shards(inputs)
    if _NC_CACHE is None:
        _NC_CACHE = build_nc()
    nc = _NC_CACHE
    from concourse import bass_utils
    res = bass_utils.run_bass_kernel_spmd(nc, shards, core_ids=list(range(NCORES)))
    bp = np.asarray(inputs["bp"], np.float32)
    outs = []
    for r in res.results:
        o = np.asarray(r["out"], dtype=np.float32)
        outs.append(o[0:N] + o[N:2 * N])
    full = np.stack([outs[2 * b] + outs[2 * b + 1] + bp[None, :]
                     for b in range(B)])
    return full.astype(np.float32)
